# revision 9
# baseline (speedup 1.0000x reference)
"""Trainium2 Bass kernel for nn_M10bTranslationAdapter (cross-attention adapter).

Reference computation (B=4, L=4096, S=10, H=2048):
    q = h_english @ w_q.T; k = h_lojban @ w_k.T; v = h_lojban @ w_v.T
    probs = softmax(q @ k.T / sqrt(H)); out = h_english + alpha * ((probs @ v) @ w_o.T)

Key re-association (S=10 is tiny, so fold the big projections through S):
    scores = h_english @ kq.T / sqrt(H),  kq = (h_lojban @ w_k.T) @ w_q   [B,S,H]
    delta  = probs @ vo,                  vo = (h_lojban @ w_v.T) @ w_o.T [B,S,H]
This removes both [16384,2048]x[2048,2048] matmuls (~275 GFLOP -> ~2.7 GFLOP),
making the problem purely HBM-bound: read h_english once, write out once.

Distribution over 8 cores:
  - h_english row-sharded (2048 rows/core; each core's rows live in one batch).
  - The kq/vo prep contraction is sharded 8-way over the 2048-wide contraction
    dim (each core reads a 256-row slice of each weight, host-packed so every
    DMA is partition-contiguous), partials combined with an on-device
    ReduceScatter whose 8 slices are laid out so core i receives exactly the
    kq/vo of its batch (i//2).
  - Main loop per 128-row tile: cast h to bf16 (DVE), transpose via the DMA
    xbar, bf16 matmuls (fp32 streams ~4x slower through the PE) accumulating
    in fp32 PSUM: scores^T = kq_T.T @ h^T, exp on ScalarE with 1/sqrt(H)
    folded into the activation scale, delta = exp_sT.T @ vo with an extra
    1/alpha column giving the softmax denominator for free, then a single
    fused DVE op per half: out = delta * (alpha/sum) + h (fp32 residual).
"""
import contextlib

import numpy as np

import concourse.tile as tile
from concourse import bacc, mybir
from concourse.bass_utils import run_bass_kernel_spmd

H = 2048
B, L, S = 4, 4096, 10
N_CORES = 8
RPC = (B * L) // N_CORES          # rows of h_english per core = 2048
OS = H // N_CORES                 # per-core weight contraction slice = 256
SB = B * S                        # flattened (batch, s) = 40
NT = RPC // 128                   # 128-row tiles per core = 16
NH = H // 128                     # 128-wide h chunks = 16
F32 = mybir.dt.float32
BF16 = mybir.dt.bfloat16

KQ_SZ = 128 * NH * S              # 20480 floats per rs slice for kq
VO_SZ = S * H                     # 20480 floats per rs slice for vo
RS_SZ = KQ_SZ + VO_SZ             # 40960

AF = mybir.ActivationFunctionType
ALU = mybir.AluOpType


def build_graph():
    # Bacc (not raw Bass): its compile() runs generate_event_semaphores(),
    # which splits multi-sem-wait instructions down to the TRN2 constraint
    # of one wait per instruction — the walrus here hard-errors otherwise.
    nc = bacc.Bacc(None, num_devices=N_CORES)

    h_in = nc.declare_dram_parameter("h_in", [RPC, H], F32, isOutput=False)
    # all prep tensors host-packed to [128, x] so DMAs are partition-contiguous
    hl_p = nc.declare_dram_parameter("hl_p", [128, NH * SB], F32, isOutput=False)
    w_kT_p = nc.declare_dram_parameter("w_kT_p", [128, NH * OS], F32, isOutput=False)
    w_q_p = nc.declare_dram_parameter("w_q_p", [128, 2 * H], F32, isOutput=False)
    w_vT_p = nc.declare_dram_parameter("w_vT_p", [128, NH * OS], F32, isOutput=False)
    w_oT_p = nc.declare_dram_parameter("w_oT_p", [128, 2 * H], F32, isOutput=False)
    inv_alpha = nc.declare_dram_parameter("inv_alpha10", [S, 1], F32, isOutput=False)
    out = nc.declare_dram_parameter("out", [RPC, H], F32, isOutput=True)

    with tile.TileContext(nc) as tc, contextlib.ExitStack() as ctx:
        singles = ctx.enter_context(tc.tile_pool(name="singles", bufs=1))
        kq_T = singles.tile([128, NH, S], BF16)     # kq_T[p, c, s] = kq[s, 128c+p]
        vo_aug = singles.tile([S, H + 1], BF16)     # vo rows + 1/alpha column

        # ------------------------- prep: kq, vo -------------------------
        with contextlib.ExitStack() as prep:
            wpool = prep.enter_context(tc.tile_pool(name="wpool", bufs=1))
            ppool = prep.enter_context(tc.tile_pool(name="ppool", bufs=1))
            pps = prep.enter_context(tc.tile_pool(name="pps", bufs=2, space="PSUM"))
            dpool = prep.enter_context(tc.tile_pool(name="dram", bufs=1, space="DRAM"))

            def load_bf16(param, shape, tag):
                # SWDGE casts f32->bf16 during the DMA itself
                t = wpool.tile(shape, BF16, tag=tag)
                nc.gpsimd.dma_start(
                    out=t[:], in_=param[:].rearrange("p (c j) -> p c j", c=shape[1])
                )
                return t

            w_kT = load_bf16(w_kT_p, [128, NH, OS], "wk")
            w_q = load_bf16(w_q_p, [128, 2, H], "wq")
            w_vT = load_bf16(w_vT_p, [128, NH, OS], "wv")
            w_oT = load_bf16(w_oT_p, [128, 2, H], "wo")
            hl = load_bf16(hl_p, [128, NH, SB], "hl")

            # k_T[o, sb] = sum_h w_k[o, h] * hl[sb, h]   (o in this core's slice)
            k_T = ppool.tile([128, 2, SB], BF16, tag="kT")
            v_T = ppool.tile([128, 2, SB], BF16, tag="vT")
            for w_sb, dst in ((w_kT, k_T), (w_vT, v_T)):
                for oc in range(2):
                    ps = pps.tile([128, SB], F32, tag="kv")
                    for hc in range(NH):
                        nc.tensor.matmul(
                            ps[:],
                            lhsT=w_sb[:, hc, 128 * oc : 128 * (oc + 1)],
                            rhs=hl[:, hc, :],
                            start=(hc == 0),
                            stop=(hc == NH - 1),
                        )
                    nc.vector.tensor_copy(dst[:, oc, :], ps[:])

            # kq_pT[h, b, s] = sum_{o slice} w_q[o, h] * k_T[o, (b s)]
            # (fp32 partial; batch-major free layout so the per-batch slice
            # shipped to the ReduceScatter is partition-contiguous)
            kq_pT = ppool.tile([128, B, NH, S], F32, tag="kqp")
            for hc in range(NH):
                ps = pps.tile([128, SB], F32, tag="kv")
                for oc in range(2):
                    nc.tensor.matmul(
                        ps[:],
                        lhsT=w_q[:, oc, 128 * hc : 128 * (hc + 1)],
                        rhs=k_T[:, oc, :],
                        start=(oc == 0),
                        stop=(oc == 1),
                    )
                nc.vector.tensor_copy(
                    kq_pT[:, :, hc, :], ps[:].rearrange("p (b s) -> p b s", b=B)
                )

            # vo_p[sb, o] = sum_{h slice} v_T[h, sb] * w_oT[h, o]  (fp32 partial)
            vo_p = ppool.tile([SB, H], F32, tag="vop")
            for n4 in range(4):
                ps = pps.tile([SB, 512], F32, tag="vo")
                for hc2 in range(2):
                    nc.tensor.matmul(
                        ps[:],
                        lhsT=v_T[:, hc2, :],
                        rhs=w_oT[:, hc2, 512 * n4 : 512 * (n4 + 1)],
                        start=(hc2 == 0),
                        stop=(hc2 == 1),
                    )
                nc.vector.tensor_copy(vo_p[:, 512 * n4 : 512 * (n4 + 1)], ps[:])

            # ReduceScatter: slice j = (kq_pT, vo_p) of batch j//2, so core i
            # receives the fully-summed kq/vo of its own batch.
            rs_in = dpool.tile([N_CORES, RS_SZ], F32, tag="rsi")
            rs_out = dpool.tile([1, RS_SZ], F32, tag="rso")
            for j in range(N_CORES):
                b = j // 2
                nc.scalar.dma_start(
                    out=rs_in[j, :KQ_SZ].rearrange("(p c s) -> p c s", p=128, c=NH),
                    in_=kq_pT[:, b, :, :],
                )
                nc.scalar.dma_start(
                    out=rs_in[j, KQ_SZ:].rearrange("(s o) -> s o", s=S),
                    in_=vo_p[S * b : S * (b + 1), :],
                )
            nc.gpsimd.collective_compute(
                "ReduceScatter",
                mybir.AluOpType.add,
                replica_groups=[list(range(N_CORES))],
                ins=[rs_in[:].opt()],
                outs=[rs_out[:].opt()],
            )
            # cast-read the scattered slice back (SWDGE f32->bf16)
            nc.gpsimd.dma_start(
                out=kq_T[:],
                in_=rs_out[0, :KQ_SZ].rearrange("(p c s) -> p c s", p=128, c=NH),
            )
            nc.gpsimd.dma_start(
                out=vo_aug[:, :H],
                in_=rs_out[0, KQ_SZ:].rearrange("(s o) -> s o", s=S),
            )
            nc.gpsimd.dma_start(out=vo_aug[:, H : H + 1], in_=inv_alpha[:])

        # ------------------------- main loop -------------------------
        with contextlib.ExitStack() as main:
            hpool = main.enter_context(tc.tile_pool(name="hpool", bufs=8))
            tpool = main.enter_context(tc.tile_pool(name="tpool", bufs=2))
            xpool = main.enter_context(tc.tile_pool(name="xpool", bufs=12))
            opool = main.enter_context(tc.tile_pool(name="opool", bufs=3))
            spool = main.enter_context(tc.tile_pool(name="spool", bufs=4))
            pp_s = main.enter_context(tc.tile_pool(name="pp_s", bufs=3, space="PSUM"))
            pp_d = main.enter_context(tc.tile_pool(name="pp_d", bufs=2, space="PSUM"))
            pp_d2 = main.enter_context(tc.tile_pool(name="pp_d2", bufs=1, space="PSUM"))

            # stage 1 for every tile: load (SP ring), cast f32->bf16 and xbar
            # transpose (both on the ACT ring) — all independent of the
            # collective, so they fill the ReduceScatter latency.
            h_ts, hTs = [], []
            for t in range(NT):
                h_t = hpool.tile([128, H], F32, tag="h")
                nc.sync.dma_start(out=h_t[:], in_=h_in[128 * t : 128 * (t + 1), :])
                h_bf = tpool.tile([128, H], BF16, tag="hbf")
                nc.vector.tensor_copy(h_bf[:], h_t[:])
                hT = xpool.tile([128, NH, 128], BF16, tag="hT")
                nc.sync.dma_start(out=hT[:], in_=h_bf[:], transpose=True)
                h_ts.append(h_t)
                hTs.append(hT)

            for t in range(NT):
                h_t = h_ts[t]
                hT = hTs[t]
                ps_s = pp_s.tile([S, 128], F32, tag="s")
                for hc in range(NH):
                    nc.tensor.matmul(
                        ps_s[:],
                        lhsT=kq_T[:, hc, :],
                        rhs=hT[:, hc, :],
                        start=(hc == 0),
                        stop=(hc == NH - 1),
                    )

                exp_sT = spool.tile([S, 128], BF16, tag="exp")
                nc.scalar.activation(
                    exp_sT[:], ps_s[:], AF.Exp, scale=float(1.0 / np.sqrt(H))
                )

                ps_d2 = pp_d2.tile([128, 1], F32, tag="d2")
                nc.tensor.matmul(
                    ps_d2[:], lhsT=exp_sT[:], rhs=vo_aug[:, H : H + 1],
                    start=True, stop=True,
                )
                r_scale = spool.tile([128, 1], F32, tag="rs")
                nc.vector.reciprocal(r_scale[:], ps_d2[:])

                out_t = opool.tile([128, H], F32, tag="out")
                for half in range(2):
                    ps_d = pp_d.tile([128, 1024], F32, tag="d")
                    for q in range(2):
                        n4 = 2 * half + q
                        nc.tensor.matmul(
                            ps_d[:, 512 * q : 512 * (q + 1)],
                            lhsT=exp_sT[:],
                            rhs=vo_aug[:, 512 * n4 : 512 * (n4 + 1)],
                            start=True,
                            stop=True,
                        )
                    # out = delta * (alpha / sum exp) + h, fused on DVE
                    nc.vector.scalar_tensor_tensor(
                        out_t[:, 1024 * half : 1024 * (half + 1)],
                        ps_d[:],
                        r_scale[:],
                        h_t[:, 1024 * half : 1024 * (half + 1)],
                        op0=ALU.mult,
                        op1=ALU.add,
                    )
                nc.gpsimd.dma_start(
                    out=out[128 * t : 128 * (t + 1), :], in_=out_t[:]
                )

    nc.compile()
    return nc


_graph_cache = {}


def _get_graph():
    if "nc" not in _graph_cache:
        _graph_cache["nc"] = build_graph()
    return _graph_cache["nc"]


def _pack(x):
    """[C*128, J] -> [128, C*J] partition-major packing (f32, contiguous)."""
    c = x.shape[0] // 128
    return np.ascontiguousarray(
        x.reshape(c, 128, x.shape[1]).transpose(1, 0, 2).reshape(128, -1)
    )


def _make_in_maps(inputs):
    h_english = np.ascontiguousarray(np.asarray(inputs["h_english"], dtype=np.float32))
    h_lojban = np.ascontiguousarray(np.asarray(inputs["h_lojban"], dtype=np.float32))
    w_q = np.asarray(inputs["w_q"], dtype=np.float32)
    w_k = np.asarray(inputs["w_k"], dtype=np.float32)
    w_v = np.asarray(inputs["w_v"], dtype=np.float32)
    w_o = np.asarray(inputs["w_o"], dtype=np.float32)
    alpha = float(np.asarray(inputs["alpha"], dtype=np.float32))

    h_flat = h_english.reshape(B * L, H)
    hl_p = _pack(np.ascontiguousarray(h_lojban.reshape(SB, H).T))
    inv_a = np.full((S, 1), 1.0 / alpha, dtype=np.float32)

    in_maps = []
    for i in range(N_CORES):
        sl = slice(OS * i, OS * (i + 1))
        in_maps.append({
            "h_in": np.ascontiguousarray(h_flat[RPC * i : RPC * (i + 1)]),
            "hl_p": hl_p,
            "w_kT_p": _pack(np.ascontiguousarray(w_k[sl, :].T)),
            "w_q_p": _pack(np.ascontiguousarray(w_q[sl, :])),
            "w_vT_p": _pack(np.ascontiguousarray(w_v[sl, :].T)),
            "w_oT_p": _pack(np.ascontiguousarray(w_o[:, sl].T)),
            "inv_alpha10": inv_a,
        })
    return in_maps


def kernel(**inputs):
    in_maps = _make_in_maps(inputs)
    nc = _get_graph()
    res = run_bass_kernel_spmd(nc, in_maps, core_ids=list(range(N_CORES)))
    out = np.concatenate([res.results[i]["out"] for i in range(N_CORES)], axis=0)
    return np.ascontiguousarray(out.reshape(B, L, H).astype(np.float32))
